# revision 20
# baseline (speedup 1.0000x reference)
"""Causal multi-head attention (B=2, S=2048, D=1024, H=16, Dh=64) on 8 trn2 cores.

Sharding: (batch, head-group) tensor parallel. Core c handles batch c//4 and
heads [4*(c%4), 4*(c%4)+4). Each core computes its 4 heads end-to-end
(QKV projections, causal softmax attention, W_O projection) and returns a
partial [S, D] bf16 output; the host sums the 4 partials per batch.

Per-core dataflow (v2):
  - Q^T, K^T produced in [Dh, S] layout so scores come out transposed
    (S^T[k, q]) and the softmax'd P~ needs no transpose for the P@V matmul.
  - Softmax denominator via a ones-column appended to V (M=65 matmuls):
    row 64 of the attention PSUM is the denominator.
  - Causal mask = multiplicative upper-tri on the exp'd bf16 tiles (GpSimd).

Engine assignment (v2 rebalance; ACT exp is the attention-phase bottleneck
at 1 elem/lane/cycle @1.2GHz, and HBM-write DMA throttles PE to half clock):
  - ACT runs ONLY exp (one table load, no swaps). All PSUM drains move to
    DVE; den row pulls / partition_broadcast / masks / half the out copies
    on GpSimd; reciprocal via DVE reciprocal_approx_fast (~5x reciprocal()).
  - QKV chains for quarter q+1 and W_O thunks for q-1 are interleaved into
    attention(q)'s kt loop: PE fills the ACT-bound exp windows instead of
    idling, and the QKV drains no longer collide with normalize on ACT.
  - Output stored bf16 (halves the HBM-write window that HAM-throttles PE);
    at/W_O stationary operands bf16 (f32r LDWEIGHTS was ~3x slower).
  - Initial loads issue from sync+scalar+gpsimd queues in parallel
    (~600ns/issue of serial descriptor-gen on one queue otherwise).
"""

import numpy as np

try:
    import concourse  # noqa: F401
except ImportError:  # pragma: no cover - harness containers stage it here
    import sys

    sys.path.insert(0, "/opt/trn_rl_repo")

B, S, D, H, DH = 2, 2048, 1024, 16, 64
NCORES = 8
HPC = 4  # heads per core
NPAIR = 2  # head pairs per core
SC = 512  # q-chunk width (scores matmul N)
NQC = S // SC  # 4 q-chunks
NST = S // 128  # 16 s/k/q tiles of 128
NDC = D // 128  # 8 contraction chunks of 128
VO_W = 65  # V columns + ones column
VO_QSTRIDE = 4 * VO_W  # per-head stride inside one quarter's V|ones tile

_cache = {}


def _build_program():
    from contextlib import ExitStack

    import concourse.mybir as mybir
    import concourse.tile as tile
    from concourse import bacc

    f32 = mybir.dt.float32
    bf16 = mybir.dt.bfloat16
    AF = mybir.ActivationFunctionType

    nc = bacc.Bacc(
        "TRN2", debug=False, target_bir_lowering=False, num_devices=NCORES
    )

    xT = nc.dram_tensor("xT", [128, NQC * NDC * SC], bf16, kind="ExternalInput").ap()
    wqk = nc.dram_tensor(
        "wqk", [128, 4 * NDC * 128], bf16, kind="ExternalInput"
    ).ap()
    wv = nc.dram_tensor("wv", [128, NDC * 256], bf16, kind="ExternalInput").ap()
    wo = nc.dram_tensor("wo", [128, NPAIR * D], bf16, kind="ExternalInput").ap()
    tri = nc.dram_tensor("tri", [128, 128], bf16, kind="ExternalInput").ap()
    out = nc.dram_tensor("out", [S, D], bf16, kind="ExternalOutput").ap()

    with tile.TileContext(nc) as tc, ExitStack() as ctx:
        persist = ctx.enter_context(tc.tile_pool(name="persist", bufs=1))
        pt_pool = ctx.enter_context(tc.tile_pool(name="pt", bufs=8))
        den_pool = ctx.enter_context(tc.tile_pool(name="den", bufs=2))
        out_pool = ctx.enter_context(tc.tile_pool(name="outsb", bufs=4))
        ps_pool = ctx.enter_context(tc.tile_pool(name="ps", bufs=2, space="PSUM"))
        pa_pool = ctx.enter_context(tc.tile_pool(name="pa", bufs=2, space="PSUM"))

        # ---- persistent SBUF tensors (per s-quarter where it matters) ----
        x_sb = {
            q: persist.tile([128, NDC * SC], bf16, tag=f"x{q}", name=f"x{q}")
            for q in range(1, NQC)
        }
        x0a_sb = persist.tile([128, SC], bf16, tag="x0a", name="x0a")
        x0b_sb = persist.tile([128, 3 * SC], bf16, tag="x0b", name="x0b")
        x0c_sb = persist.tile([128, 4 * SC], bf16, tag="x0c", name="x0c")

        def x_slice(q, dc, lo=0, hi=SC):
            if q == 0:
                if dc == 0:
                    return x0a_sb[:, lo:hi]
                if dc < 4:
                    return x0b_sb[:, (dc - 1) * SC + lo : (dc - 1) * SC + hi]
                return x0c_sb[:, (dc - 4) * SC + lo : (dc - 4) * SC + hi]
            return x_sb[q][:, dc * SC + lo : dc * SC + hi]

        wqk_sb = persist.tile([128, 4 * NDC * 128], bf16, tag="wqk", name="wqk_sb")
        wv_sb = persist.tile([128, NDC * 256], bf16, tag="wv", name="wv_sb")
        wo_sb = persist.tile([128, NPAIR * D], bf16, tag="wo", name="wo_sb")
        trib_sb = persist.tile([128, 128], bf16, tag="trib", name="trib_sb")
        ones_sb = persist.tile([128, 1], f32, tag="ones", name="ones_sb")
        qt_sb = {
            (p, q): persist.tile([128, SC], bf16, tag=f"qt{p}_{q}", name=f"qt{p}_{q}")
            for p in range(NPAIR)
            for q in range(NQC)
        }
        kt_sb = {
            (p, q): persist.tile([128, SC], bf16, tag=f"kt{p}_{q}", name=f"kt{p}_{q}")
            for p in range(NPAIR)
            for q in range(NQC)
        }
        vo_sb = {
            q: persist.tile(
                [128, HPC * VO_QSTRIDE], bf16, tag=f"vo{q}", name=f"vo{q}"
            )
            for q in range(NQC)
        }
        at_sb = {
            (p, qc): persist.tile(
                [128, SC], bf16, tag=f"at{p}_{qc}", name=f"at{p}_{qc}"
            )
            for p in range(NPAIR)
            for qc in range(NQC)
        }

        # ---- loads: two HWDGE queues (sync+scalar) issued in NEED order so
        # the shared DMA-engine pool serves the critical QKV(0) data first ----
        BW = NDC * 128
        nc.sync.dma_start(x0a_sb[:], xT[:, 0:SC])
        nc.scalar.dma_start(wqk_sb[:, 0:BW], wqk[:, 0:BW])
        nc.sync.dma_start(x0b_sb[:], xT[:, SC : 4 * SC])
        nc.scalar.dma_start(x0c_sb[:], xT[:, 4 * SC : NDC * SC])
        nc.sync.dma_start(
            wqk_sb[:, 2 * BW : 3 * BW], wqk[:, 2 * BW : 3 * BW]
        )
        nc.scalar.dma_start(wqk_sb[:, BW : 2 * BW], wqk[:, BW : 2 * BW])
        nc.scalar.dma_start(
            wqk_sb[:, 3 * BW : 4 * BW], wqk[:, 3 * BW : 4 * BW]
        )
        nc.sync.dma_start(wv_sb[:], wv[:])
        nc.scalar.dma_start(trib_sb[:], tri[:])
        nc.sync.dma_start(x_sb[1][:], xT[:, NDC * SC : 2 * NDC * SC])
        nc.scalar.dma_start(x_sb[2][:], xT[:, 2 * NDC * SC : 3 * NDC * SC])
        nc.sync.dma_start(x_sb[3][:], xT[:, 3 * NDC * SC : 4 * NDC * SC])
        nc.scalar.dma_start(wo_sb[:], wo[:])
        nc.vector.memset(ones_sb[:], 1.0)
        for q in range(NQC):
            ones_cols = vo_sb[q].rearrange(
                "p (h s w) -> p h s w", h=HPC, w=VO_W
            )[:, :, :, 64]
            nc.vector.tensor_copy(
                ones_cols, ones_sb[:].to_broadcast((128, HPC, 4))
            )

        def qkv_units(q):
            """8 PE chain units (4 QK + 4 V); drains on DVE."""
            units = []
            for p in range(NPAIR):
                for qk, dst in ((0, qt_sb[(p, q)]), (1, kt_sb[(p, q)])):
                    def qk_unit(p=p, qk=qk, dst=dst, q=q):
                        ps = ps_pool.tile(
                            [128, SC], f32, tag="ps", name=f"psqk{p}{qk}{q}"
                        )
                        for dc in range(NDC):
                            col = ((qk * NPAIR + p) * NDC + dc) * 128
                            nc.tensor.matmul(
                                ps[:, 0:SC],
                                lhsT=wqk_sb[:, col : col + 128],
                                rhs=x_slice(q, dc),
                                start=(dc == 0),
                                stop=(dc == NDC - 1),
                            )
                        nc.scalar.copy(dst[:], ps[:, 0:SC])
                    units.append(qk_unit)
            for st4 in range(4):
                def v_unit(st4=st4, q=q):
                    ps = ps_pool.tile([128, 256], f32, tag="ps", name=f"psv{q}{st4}")
                    for dc in range(NDC):
                        nc.tensor.matmul(
                            ps[:],
                            lhsT=x_slice(q, dc, st4 * 128, (st4 + 1) * 128),
                            rhs=wv_sb[:, dc * 256 : (dc + 1) * 256],
                            start=(dc == 0),
                            stop=(dc == NDC - 1),
                        )
                    vo_cols = vo_sb[q].rearrange(
                        "p (h s w) -> p h s w", h=HPC, w=VO_W
                    )[:, :, st4, 0:64]
                    nc.scalar.copy(
                        vo_cols, ps[:].rearrange("p (h e) -> p h e", e=64)
                    )
                units.append(v_unit)
            return units

        def wo_units(qc, act_copy=False):
            # one consolidated [128, 4, D] bf16 tile per chunk; the 1MB store
            # is returned separately so ALL stores run in the tail where the
            # HBM-write HAM throttle hits an idle PE. act_copy puts half the
            # PSUM drains on ACT when its window has slack.
            outc = out_pool.tile([128, 4, D], bf16, tag="outsb", name=f"oc{qc}")
            units = []
            for qt in range(4):
                def wo_unit(qc=qc, qt=qt):
                    # single 2-bank tile so a WO unit holds one ps slot, not
                    # both (keeps the next scores matmul's slot free)
                    po = ps_pool.tile([128, D], f32, tag="ps", name=f"po{qc}{qt}")
                    for p in range(NPAIR):
                        for dc in range(2):
                            nc.tensor.matmul(
                                po[:, dc * SC : (dc + 1) * SC],
                                lhsT=at_sb[(p, qc)][:, qt * 128 : (qt + 1) * 128],
                                rhs=wo_sb[:, p * D + dc * SC : p * D + (dc + 1) * SC],
                                start=(p == 0),
                                stop=(p == NPAIR - 1),
                            )
                    if act_copy:
                        nc.scalar.copy(outc[:, qt, 0:SC], po[:, 0:SC])
                    else:
                        nc.vector.tensor_copy(outc[:, qt, 0:SC], po[:, 0:SC])
                    nc.vector.tensor_copy(outc[:, qt, SC:D], po[:, SC:D])
                units.append(wo_unit)

            def store(qc=qc, outc=outc):
                row = qc * SC
                nc.sync.dma_start(
                    out[row : row + SC, :].rearrange("(t p) d -> p t d", p=128),
                    outc[:],
                )
            return units, store

        def emit_attention(qc, extras=()):
            """kt-loop with `extras` (PE-heavy closures) interleaved so the
            tensor engine fills ACT-bound exp windows."""
            extras = list(extras)
            nkt = 4 * (qc + 1)
            pa_qc = {
                p: pa_pool.tile([VO_W, 2 * SC], f32, tag="pa", name=f"pa{qc}{p}")
                for p in range(NPAIR)
            }

            def flush(p, kt, ptile):
                j0 = max(0, kt * 128 - qc * SC)
                kq, kst = kt // 4, kt % 4
                for par in range(2):
                    hh = 2 * p + par
                    vbase = hh * VO_QSTRIDE + kst * VO_W
                    nc.tensor.matmul(
                        pa_qc[p][:, par * SC + j0 : (par + 1) * SC],
                        lhsT=vo_sb[kq][:, vbase : vbase + VO_W],
                        rhs=ptile[:, par * SC + j0 : (par + 1) * SC],
                        start=(kt == 0),
                        stop=(kt == nkt - 1),
                    )

            # both pairs advance kt together: two independent
            # scores->exp->attn chains keep ACT continuously fed
            pending = []  # (p, kt, ptile) awaiting the P@V matmul
            # spread extras across the kt loop
            ei = 0
            for kt in range(nkt):
                j0 = max(0, kt * 128 - qc * SC)
                kq, kst = kt // 4, kt % 4
                for p in range(NPAIR):
                    ps_s = ps_pool.tile(
                        [128, 2 * SC], f32, tag="ps", name=f"pss{qc}{p}{kt}"
                    )
                    for par in range(2):
                        nc.tensor.matmul(
                            ps_s[:, par * SC + j0 : (par + 1) * SC],
                            lhsT=kt_sb[(p, kq)][
                                par * 64 : (par + 1) * 64,
                                kst * 128 : (kst + 1) * 128,
                            ],
                            rhs=qt_sb[(p, qc)][par * 64 : (par + 1) * 64, j0:SC],
                            start=True,
                            stop=True,
                        )
                    ptile = pt_pool.tile(
                        [128, 2 * SC], bf16, tag="pt", name=f"pt{qc}{p}{kt}"
                    )
                    nc.scalar.activation(
                        ptile.rearrange("p (b n) -> p b n", b=2)[:, :, j0:SC],
                        ps_s.rearrange("p (b n) -> p b n", b=2)[:, :, j0:SC],
                        AF.Exp,
                        scale=0.125,
                    )
                    if kt * 128 >= qc * SC:  # diagonal blocks: causal mask on
                        # the exp'd bf16 tile (multiplicative, on idle GpSimd)
                        nc.gpsimd.tensor_mul(
                            ptile.rearrange("p (b n) -> p b n", b=2)[
                                :, :, j0 : j0 + 128
                            ],
                            ptile.rearrange("p (b n) -> p b n", b=2)[
                                :, :, j0 : j0 + 128
                            ],
                            trib_sb[:].unsqueeze(1).to_broadcast((128, 2, 128)),
                        )
                    pending.append((p, kt, ptile))
                while len(pending) > 6:
                    flush(*pending.pop(0))
                # interleave extra PE units evenly across the kt loop
                want = (kt + 1) * len(extras) // nkt
                while ei < want:
                    extras[ei]()
                    ei += 1
            while ei < len(extras):
                extras[ei]()
                ei += 1
            for pend in pending:
                flush(*pend)
            return pa_qc

        def emit_normalize(qc, pa_qc, act_pull=False):
            # den path off ACT (DVE pull + fast-recip, GpSimd broadcast, DVE
            # multiply) except act_pull=True (tail: ACT is idle and shortens
            # the serial chain).
            for p in range(NPAIR):
                for par in range(2):
                    den_a = den_pool.tile(
                        [1, SC], f32, tag="dena", name=f"dena{qc}{p}{par}"
                    )
                    pull = nc.scalar.copy if act_pull else nc.vector.tensor_copy
                    pull(
                        den_a[:], pa_qc[p][64:65, par * SC : (par + 1) * SC]
                    )
                    den_r = den_pool.tile(
                        [1, SC], f32, tag="denr", name=f"denr{qc}{p}{par}"
                    )
                    nc.vector.reciprocal_approx_fast(den_r[:], den_a[:])
                    denb = den_pool.tile(
                        [64, SC], f32, tag="denb", name=f"denb{qc}{p}{par}"
                    )
                    nc.gpsimd.partition_broadcast(denb[:], den_r[:])
                    nc.vector.tensor_mul(
                        at_sb[(p, qc)][par * 64 : (par + 1) * 64, :],
                        pa_qc[p][0:64, par * SC : (par + 1) * SC],
                        denb[:],
                    )

        pa_prev = None
        stores = []
        for q in range(NQC):
            if q == 0:
                for u in qkv_units(0):
                    u()
            if q >= 1:
                emit_normalize(q - 1, pa_prev)
            extras = []
            if q + 1 < NQC:
                extras += qkv_units(q + 1)
            if q >= 1:
                # wo(q-1) drains on ACT only where its window (attn(q)) has
                # ACT slack; attn(3) is exp-bound so wo(2) stays on DVE
                units, store = wo_units(q - 1, act_copy=(q < 3))
                extras += units
                stores.append(store)
            pa_prev = emit_attention(q, extras)
        emit_normalize(NQC - 1, pa_prev, act_pull=True)
        for st in stores:  # tail: HBM writes throttle only an idle PE here
            st()
        units, store = wo_units(NQC - 1, act_copy=True)
        for u in units:
            u()
        store()

    nc.compile()
    return nc


def _get_program():
    if "nc" not in _cache:
        _cache["nc"] = _build_program()
    return _cache["nc"]


def _prep_core_inputs(c, residual, W_Q, W_K, W_V, W_O, tri):
    import ml_dtypes

    b = c // 4
    heads = [4 * (c % 4) + i for i in range(HPC)]

    def chunked(w):  # [1024, M] -> [128, NDC*M] chunk-major
        m = w.shape[1]
        return np.ascontiguousarray(
            w.reshape(NDC, 128, m).transpose(1, 0, 2).reshape(128, NDC * m)
        )

    wqk_blocks = []
    for Wt in (W_Q, W_K):
        for p in range(NPAIR):
            h0, h1 = heads[2 * p], heads[2 * p + 1]
            wpair = np.concatenate([Wt[h0].T, Wt[h1].T], axis=1)  # [1024, 128]
            wqk_blocks.append(chunked(wpair))
    wqk_arr = np.ascontiguousarray(np.concatenate(wqk_blocks, axis=1))

    wv_arr = chunked(np.concatenate([W_V[h].T for h in heads], axis=1))
    wo_arr = np.ascontiguousarray(
        np.concatenate(
            [
                np.concatenate([W_O[heads[2 * p]], W_O[heads[2 * p + 1]]], axis=0)
                for p in range(NPAIR)
            ],
            axis=1,
        )
    )
    xt = residual[b].T.astype(ml_dtypes.bfloat16)  # [1024, 2048]
    xq = np.concatenate(
        [
            np.concatenate(
                [xt[dc * 128 : (dc + 1) * 128, q * SC : (q + 1) * SC]
                 for dc in range(NDC)], axis=1)
            for q in range(NQC)
        ],
        axis=1,
    )
    return {
        "xT": np.ascontiguousarray(xq),
        "wqk": wqk_arr.astype(ml_dtypes.bfloat16),
        "wv": wv_arr.astype(ml_dtypes.bfloat16),
        "wo": wo_arr.astype(ml_dtypes.bfloat16),
        "tri": tri,
    }


def make_in_maps(residual, W_Q, W_K, W_V, W_O):
    residual = np.asarray(residual, np.float32)
    W_Q, W_K, W_V, W_O = (np.asarray(w, np.float32) for w in (W_Q, W_K, W_V, W_O))
    import ml_dtypes

    # multiplicative causal mask for S^T[k, q] diagonal blocks: keep j >= p
    tri = np.triu(np.ones((128, 128), np.float32)).astype(ml_dtypes.bfloat16)
    return [
        _prep_core_inputs(c, residual, W_Q, W_K, W_V, W_O, tri)
        for c in range(NCORES)
    ]


def gather(results):
    out = np.zeros((B, S, D), np.float64)
    for c in range(NCORES):
        out[c // 4] += results[c]["out"].astype(np.float64)
    return out.astype(np.float32)


def kernel(residual, W_Q, W_K, W_V, W_O, **run_kwargs):
    from concourse.bass_utils import run_bass_kernel_spmd

    nc = _get_program()
    in_maps = make_in_maps(residual, W_Q, W_K, W_V, W_O)
    res = run_bass_kernel_spmd(nc, in_maps, core_ids=list(range(NCORES)), **run_kwargs)
    out = gather(res.results)
    if run_kwargs:
        _cache["last_results"] = res
    return out
